# revision 1
# baseline (speedup 1.0000x reference)
"""Trainium2 Bass kernel for nn_CausalSelfAttention_6442450944521.

Sparse-attention causal self-attention block:
  B=4, T=2048 (rows<512: full attention over cols<512; rows>=512: causal),
  E=1024, H=16, D=64.

Sharding: batch (4) x head-group (2 groups of 8 heads) across 8 cores.
Each core computes, for its (batch b, head-group g):
  qkv^T projections (Q^T,K^T in [D,T] layout; V in natural [T,D] layout),
  block-sparse attention via S^T = K Q^T tiles (softmax denominators come
  free from a ones-column packed next to V), and its row-slice of the
  output projection. The two head-group partials per batch are summed on
  the host (row-parallel tensor parallelism); v-bias and proj-bias are
  folded in exactly on the host.

Matmul operands are bf16 (1 cyc/row on the PE); accumulation is fp32 in
PSUM; softmax denominators and normalization are fp32. All stationary
operands are zero-padded to full 128x128 — half-utilization matmuls read
as "idle" to the PE activity monitor, which then throttles the PE clock
to 1.2 GHz. V is computed first, then per head-pair Q/K immediately
followed by that pair's two attention heads, so the scheduler overlaps
the next pair's projections with attention.
"""

import os
import sys

if "/opt/trn_rl_repo" not in sys.path:
    sys.path.insert(0, "/opt/trn_rl_repo")

import numpy as np

# Problem constants (hardcoded per harness contract).
B = 4
T = 2048
E = 1024
H = 16
D = 64
NCORES = 8
HPC = H // 2          # heads per core = 8
ESL = HPC * D         # per-core E-slice = 512
P = 128               # SBUF/PSUM partitions
TG = 512              # matmul moving-dim tile (q-group width)
NTG = T // TG         # 4
NTT = T // P          # 16
NEC = E // P          # 8 contraction chunks over E
NPAIR = HPC // 2      # 4 head-pair tiles

_CACHE = {}


def _build_program():
    import concourse.bass as bass
    import concourse.tile as tile
    from concourse import bacc, mybir

    f32 = mybir.dt.float32
    bf16 = mybir.dt.bfloat16

    nc = bacc.Bacc("TRN2", target_bir_lowering=False, debug=False,
                   num_devices=NCORES)

    xT = nc.dram_tensor("xT", [E, T], bf16, kind="ExternalInput").ap()
    wq = nc.dram_tensor("wq", [E, ESL], bf16, kind="ExternalInput").ap()
    wk = nc.dram_tensor("wk", [E, ESL], bf16, kind="ExternalInput").ap()
    wv = nc.dram_tensor("wv", [E, ESL], bf16, kind="ExternalInput").ap()
    wp = nc.dram_tensor("wp", [ESL, E], bf16, kind="ExternalInput").ap()
    bq = nc.dram_tensor("bq", [ESL, 1], f32, kind="ExternalInput").ap()
    bk = nc.dram_tensor("bk", [ESL, 1], f32, kind="ExternalInput").ap()
    trimask = nc.dram_tensor("trimask", [P, 3 * TG], bf16,
                             kind="ExternalInput").ap()
    out = nc.dram_tensor("out", [T, E], f32, kind="ExternalOutput").ap()

    with tile.TileContext(nc) as tc:
        _body(nc, tc, tile, mybir, bass,
              xT, wq, wk, wv, wp, bq, bk, trimask, out)

    nc.compile()
    return nc


def _body(nc, tc, tile, mybir, bass,
          xT, wq, wk, wv, wp, bq, bk, trimask, out):
    f32 = mybir.dt.float32
    bf16 = mybir.dt.bfloat16
    Exp = mybir.ActivationFunctionType.Exp

    cms = {}

    def open_pool(name, bufs, space=None, side=None):
        kw = {}
        if space:
            kw["space"] = space
        if side:
            kw["side"] = side
        cm = tc.tile_pool(name=name, bufs=bufs, **kw)
        pool = cm.__enter__()
        cms[id(pool)] = cm
        return pool

    def close_pool(pool):
        cms.pop(id(pool)).__exit__(None, None, None)

    # ---- pools ----------------------------------------------------------
    singles = open_pool("singles", 1)
    yT_pool = open_pool("yTpool", 1)
    ps_all = open_pool("ps", 2, space="PSUM")        # per-tile bufs override
    xr_pool = open_pool("xr", 1)                     # resident x^T (bf16)
    w_pool = open_pool("w", 1)                       # resident weights
    pT_pool = open_pool("pT", 6)
    ob_pool = open_pool("ob", 5)
    rc_pool = open_pool("rc", 2)
    bc_pool = open_pool("bc", 2)
    on_pool = open_pool("on", 2)
    dr_pool = open_pool("dr", 2, space="DRAM")
    # right-stack: big attention-phase tensors
    qk_pool = open_pool("qkpool", 1, side="right")
    v_pool = open_pool("vpool", 1, side="right")

    # ---- resident tensors ------------------------------------------------
    mask_t = singles.tile([P, 3 * TG], bf16, tag="mask", name="mask")
    bias_t = singles.tile([P, 2 * NPAIR], f32, tag="bias", name="bias")
    nc.sync.dma_start(out=mask_t[:], in_=trimask)
    for pt in range(NPAIR):
        nc.sync.dma_start(out=bias_t[:, pt:pt + 1],
                          in_=bq[pt * P:(pt + 1) * P, :])
        nc.sync.dma_start(out=bias_t[:, NPAIR + pt:NPAIR + pt + 1],
                          in_=bk[pt * P:(pt + 1) * P, :])

    wv_c = []
    for ec in range(NEC):
        t = w_pool.tile([P, ESL], bf16, tag="wv", name="wvc", bufs=NEC)
        nc.sync.dma_start(out=t[:], in_=wv[ec * P:(ec + 1) * P, :])
        wv_c.append(t)
    # x^T loaded in column chunks so the V phase starts after ~1/4 of the fill
    xr = []
    for ec in range(NEC):
        t = xr_pool.tile([P, T], bf16, tag=f"xr{ec}", name=f"xr{ec}")
        xr.append(t)
    for ch in range(4):
        c0 = ch * (T // 4)
        for ec in range(NEC):
            nc.sync.dma_start(out=xr[ec][:, c0:c0 + T // 4],
                              in_=xT[ec * P:(ec + 1) * P, c0:c0 + T // 4])

    yT_t = [yT_pool.tile([P, T], bf16, tag=f"yT{i}", name=f"yT{i}")
            for i in range(NPAIR)]
    qT_t = [qk_pool.tile([P, T], bf16, tag=f"qT{i}", name=f"qT{i}")
            for i in range(NPAIR)]
    # K^T per head, zero-padded to [128, T] (full-width PE stationary).
    kT_t = [qk_pool.tile([P, T], bf16, tag=f"kT{i}", name=f"kT{i}")
            for i in range(HPC)]
    for hh in range(HPC):
        zr = (1 - hh % 2) * 64
        nc.gpsimd.memset(kT_t[hh][zr:zr + 64, :], 0.0)
    # V per T-tile: per head [V(64) | ones | zeros(63)] = 128-col stationary.
    v_t = [v_pool.tile([P, HPC, P], bf16, tag=f"v{i}", name=f"v{i}")
           for i in range(NTT)]

    wq_c, wk_c, wp_c = {}, {}, {}

    # ---- V = x @ Wv ------------------------------------------------------
    for tt in range(NTT):
        ts_ = slice(tt * P, (tt + 1) * P)
        psv = ps_all.tile([P, HPC, D], f32, tag="qk", name="psv", bufs=2)
        for ec in range(NEC):
            nc.tensor.matmul(psv[:, :, :], lhsT=xr[ec][:, ts_],
                             rhs=wv_c[ec][:],
                             start=(ec == 0), stop=(ec == NEC - 1))
        nc.gpsimd.memset(v_t[tt][:, :, D + 1:], 0.0)
        nc.gpsimd.memset(v_t[tt][:, :, D:D + 1], 1.0)
        nc.vector.tensor_copy(v_t[tt][:, :, 0:D], psv[:, :, :])

    # ---- per pair: Q/K projections, then the pair's two heads ------------
    for pt in range(NPAIR):
        for ec in range(NEC):
            t = w_pool.tile([P, P], bf16, tag="wq", name="wqc", bufs=2 * NEC)
            nc.sync.dma_start(out=t[:], in_=wq[ec * P:(ec + 1) * P,
                                              pt * P:(pt + 1) * P])
            wq_c[(pt, ec)] = t
            t = w_pool.tile([P, P], bf16, tag="wk", name="wkc", bufs=2 * NEC)
            nc.sync.dma_start(out=t[:], in_=wk[ec * P:(ec + 1) * P,
                                              pt * P:(pt + 1) * P])
            wk_c[(pt, ec)] = t
        for tg in range(NTG):
            cs = slice(tg * TG, (tg + 1) * TG)
            psq = ps_all.tile([P, TG], f32, tag="qk", name="psq", bufs=2)
            for ec in range(NEC):
                nc.tensor.matmul(psq[:], lhsT=wq_c[(pt, ec)][:],
                                 rhs=xr[ec][:, cs],
                                 start=(ec == 0), stop=(ec == NEC - 1))
            nc.scalar.add(qT_t[pt][:, cs], psq[:], bias_t[:, pt:pt + 1])
            psk = ps_all.tile([P, TG], f32, tag="qk", name="psk", bufs=2)
            for ec in range(NEC):
                nc.tensor.matmul(psk[:], lhsT=wk_c[(pt, ec)][:],
                                 rhs=xr[ec][:, cs],
                                 start=(ec == 0), stop=(ec == NEC - 1))
            nc.scalar.add(kT_t[2 * pt][0:64, cs], psk[0:64, :],
                          bias_t[0:64, NPAIR + pt:NPAIR + pt + 1])
            nc.scalar.add(kT_t[2 * pt + 1][64:P, cs], psk[64:P, :],
                          bias_t[64:P, NPAIR + pt:NPAIR + pt + 1])

        # ---- attention for heads 2pt, 2pt+1 ----
        # S blocks are packed into [128, 1024] two-bank PSUM bins so one
        # ACTIVATE(exp) covers up to 1024 columns (the ~310ns per-op ACT
        # overhead was pacing the whole attention phase). Diagonal blocks
        # are sliced to their valid columns (block m keeps 512-m*128) and
        # in the sliced frame all diagonal masks are the same (j' >= i).
        for h in (2 * pt, 2 * pt + 1):
            rb = (h % 2) * 64
            for qg in range(NTG):
                qb = qg * TG
                # bins: list of entries (kt, s0) packed to <=1024 cols;
                # s0 = start column within the q-group (n = 512 - s0).
                nf = 4 if qg == 0 else qb // P
                bins = [[(kt, 0), (kt + 1, 0)] for kt in range(0, nf, 2)]
                if qg > 0:
                    m0 = qb // P
                    bins.append([(m0, 0), (m0 + 1, P)])          # 512+384
                    bins.append([(m0 + 2, 2 * P), (m0 + 3, 3 * P)])  # 256+128
                po = ps_all.tile([P, TG], f32, tag="o", name="po", bufs=2)
                n_mm = sum(len(b) for b in bins)
                i = 0
                for bin_ in bins:
                    for kt, s0 in bin_:
                        n = TG - s0
                        ks = slice(kt * P, (kt + 1) * P)
                        pss = ps_all.tile([P, TG], f32, tag="s", name="pss",
                                          bufs=3)
                        pT = pT_pool.tile([P, TG], bf16, tag="pT", name="pT")
                        nc.tensor.matmul(pss[:, 0:n],
                                         lhsT=kT_t[h][:, ks],
                                         rhs=qT_t[pt][:, qb + s0:qb + TG],
                                         start=True, stop=True)
                        nc.scalar.activation(pT[:, 0:n], pss[:, 0:n],
                                             Exp, scale=0.125)
                        diag = qg > 0 and kt >= qb // P
                        if diag:
                            nc.vector.tensor_mul(pT[:, 0:n], pT[:, 0:n],
                                                 mask_t[:, TG:TG + n])
                        nc.tensor.matmul(po[:, s0:TG],
                                         lhsT=v_t[kt][:, h, :],
                                         rhs=pT[:, 0:n],
                                         start=(i == 0), stop=(i == n_mm - 1))
                        i += 1
                # Evacuate rows 0..64 (O + denominator); frees the bank,
                # then normalize this q-group immediately (short tail).
                ob = ob_pool.tile([D + 1, TG], f32, tag="ob", name="ob")
                nc.vector.tensor_copy(ob[:], po[0:D + 1, :])
                # Denominator row -> DRAM -> partition-broadcast back; the
                # reciprocal runs on the [64, TG] broadcast at base
                # partition 0 (reciprocal_approx_* misbehaves off base 0).
                den_d = dr_pool.tile([1, TG], f32, tag="den_d", name="den_d")
                nc.sync.dma_start(out=den_d[:], in_=ob[D:D + 1, :])
                bcast_in = bass.AP(
                    tensor=den_d.tensor, offset=den_d.offset,
                    ap=[[0, D]] + [list(a) for a in den_d.ap[1:]])
                bc = bc_pool.tile([D, TG], f32, tag="bc", name="bc")
                nc.sync.dma_start(out=bc[:], in_=bcast_in)
                rcp = rc_pool.tile([D, TG], f32, tag="rcp", name="rcp")
                nc.vector.reciprocal_approx_fast(out=rcp[:], in_=bc[:])
                on = on_pool.tile([D, TG], bf16, tag="on", name="on")
                nc.vector.tensor_mul(on[:], ob[0:D, :], rcp[:])
                nc.sync.dma_start(
                    out=yT_t[pt][rb:rb + 64, qg * TG:(qg + 1) * TG],
                    in_=on[:])

    # ---- proj: out = y @ Wp (row-parallel partial) -----------------------
    ot_pool = open_pool("ot", 4)
    for c in range(NPAIR):
        for ng in range(E // TG):
            t = w_pool.tile([P, TG], bf16, tag="wp", name="wpc", bufs=2 * NPAIR)
            nc.sync.dma_start(out=t[:], in_=wp[c * P:(c + 1) * P,
                                              ng * TG:(ng + 1) * TG])
            wp_c[(c, ng)] = t
    for tt in range(NTT):
        ts_ = slice(tt * P, (tt + 1) * P)
        for ng in range(E // TG):
            pp = ps_all.tile([P, TG], f32, tag="qk", name="pp", bufs=2)
            for c in range(NPAIR):
                nc.tensor.matmul(pp[:], lhsT=yT_t[c][:, ts_],
                                 rhs=wp_c[(c, ng)][:],
                                 start=(c == 0), stop=(c == NPAIR - 1))
            ot = ot_pool.tile([P, TG], f32, tag="ot", name="ot")
            nc.vector.tensor_copy(ot[:], pp[:])
            nc.sync.dma_start(out=out[ts_, ng * TG:(ng + 1) * TG], in_=ot[:])

    close_pool(ot_pool)
    close_pool(v_pool)
    close_pool(qk_pool)
    close_pool(dr_pool)
    close_pool(on_pool)
    close_pool(bc_pool)
    close_pool(rc_pool)
    close_pool(ob_pool)
    close_pool(pT_pool)
    close_pool(w_pool)
    close_pool(xr_pool)
    close_pool(ps_all)
    close_pool(yT_pool)
    close_pool(singles)


def _get_program():
    if "nc" not in _CACHE:
        _CACHE["nc"] = _build_program()
    return _CACHE["nc"]


def make_in_maps(x, W_qkv, b_qkv, W_proj):
    """Per-core input dicts: core c -> (batch c%4, head-group c//4)."""
    import ml_dtypes
    x = np.asarray(x, np.float32)
    W_qkv = np.asarray(W_qkv, np.float32)
    b_qkv = np.asarray(b_qkv, np.float32)
    tri = ((np.arange(3 * TG)[None, :] - TG) >=
           np.arange(P)[:, None]).astype(np.float32)
    cvt = lambda a: np.ascontiguousarray(a).astype(ml_dtypes.bfloat16)
    in_maps = []
    for c in range(NCORES):
        b, g = c % B, c // B
        gs = slice(g * ESL, (g + 1) * ESL)
        in_maps.append({
            "xT": cvt(x[b].T),
            "wq": cvt(W_qkv[:, 0 * E:1 * E][:, gs]),
            "wk": cvt(W_qkv[:, 1 * E:2 * E][:, gs]),
            "wv": cvt(W_qkv[:, 2 * E:3 * E][:, gs]),
            "wp": cvt(np.asarray(W_proj, np.float32)[gs, :]),
            "bq": np.ascontiguousarray(b_qkv[0 * E:1 * E][gs, None]),
            "bk": np.ascontiguousarray(b_qkv[1 * E:2 * E][gs, None]),
            "trimask": cvt(tri),
        })
    return in_maps


def gather_output(results, b_qkv, b_proj, W_proj):
    """Sum the two row-parallel partials per batch; fold v/proj biases."""
    b_qkv = np.asarray(b_qkv, np.float64)
    W_proj = np.asarray(W_proj, np.float64)
    b_v = b_qkv[2 * E:3 * E]
    const = b_v @ W_proj + np.asarray(b_proj, np.float64)
    out = np.empty((B, T, E), np.float32)
    for b in range(B):
        out[b] = (results[b]["out"].astype(np.float64) +
                  results[b + B]["out"].astype(np.float64) +
                  const).astype(np.float32)
    return out


def run_on_hw(inputs, trace=False, **kwargs):
    from concourse.bass_utils import run_bass_kernel_spmd
    nc = _get_program()
    in_maps = make_in_maps(inputs["x"], inputs["W_qkv"], inputs["b_qkv"],
                           inputs["W_proj"])
    res = run_bass_kernel_spmd(nc, in_maps, list(range(NCORES)), trace=trace,
                               **kwargs)
    out = gather_output(res.results, inputs["b_qkv"], inputs["b_proj"],
                        inputs["W_proj"])
    return out, res


def kernel(x, W_qkv, b_qkv, W_proj, b_proj):
    out, _ = run_on_hw({"x": x, "W_qkv": W_qkv, "b_qkv": b_qkv,
                        "W_proj": W_proj, "b_proj": b_proj})
    return out



# revision 3
# speedup vs baseline: 1.0737x; 1.0737x over previous
"""Trainium2 Bass kernel for nn_CausalSelfAttention_6442450944521.

Sparse-attention causal self-attention block:
  B=4, T=2048 (rows<512: full attention over cols<512; rows>=512: causal),
  E=1024, H=16, D=64.

Sharding: batch (4) x head-group (2 groups of 8 heads) across 8 cores.
Each core computes qkv^T projections, block-sparse attention via S^T = K Q^T
tiles, and its row-slice of the output projection; the two head-group
partials per batch are summed on the host (row-parallel tensor parallelism).

v2 structure:
  - S^T matmuls are ROW-TILED: the two heads of a pair run concurrently on
    the 128x128 PE array (head A rows 0-63 via tile_position (0,0), head B
    rows 64-127 via (64,0)) since the contraction dim is only D=64. kT/qT
    are stored pair-stacked [128, T] so base-partition slicing infers the
    tile positions; no zero padding or memsets needed.
  - PV keeps the k-contraction layout with a 65-col stationary [V|ones]
    (ones column yields the softmax denominator for free); no 128-padding.
  - S blocks land in [128, 1024] two-bank PSUM bins; ONE exp per bin on the
    scalar engine (per-op ACT overhead was pacing the attention phase).
  - Diagonal-block masks are applied with one host-packed mask tile.
  - Q/K bias adds + PSUM evacuations run on the vector engine.
  - Softmax normalization: denominator row -> DRAM -> partition-broadcast
    back (gpsimd DMA queue), reciprocal+multiply on DVE, off the PE path.
  - Projection (V/QK/out-proj) matmul units are interleaved between
    attention bins in PE program order so the PE fills scalar-exp waits.
"""

import os
import sys

if "/opt/trn_rl_repo" not in sys.path:
    sys.path.insert(0, "/opt/trn_rl_repo")

import numpy as np

# Problem constants (hardcoded per harness contract).
B = 4
T = 2048
E = 1024
H = 16
D = 64
NCORES = 8
HPC = H // 2          # heads per core = 8
ESL = HPC * D         # per-core E-slice = 512
P = 128               # SBUF/PSUM partitions
TG = 512              # q-group width
NTG = T // TG         # 4
NTT = T // P          # 16
NEC = E // P          # 8 contraction chunks over E
NPAIR = HPC // 2      # 4 head-pair tiles

_CACHE = {}


def _att_bins(qg):
    """Bins of S^T blocks for q-group qg. Each bin is a list of
    (kt, c0, s0, n): k-tile index, column offset in the [128,1024] bin,
    q-offset within the group, and width. Total bin width <= 1024."""
    bins = []
    nf = 4 if qg == 0 else 4 * qg
    for k0 in range(0, nf, 2):
        bins.append([(k0, 0, 0, TG), (k0 + 1, TG, 0, TG)])
    if qg > 0:
        m0 = 4 * qg
        bins.append([(m0, 0, 0, 512), (m0 + 1, 512, 128, 384)])
        bins.append([(m0 + 2, 0, 256, 256), (m0 + 3, 256, 384, 128)])
    return bins


def _build_program():
    import concourse.bass as bass
    import concourse.tile as tile
    from concourse import bacc, mybir

    f32 = mybir.dt.float32
    bf16 = mybir.dt.bfloat16

    nc = bacc.Bacc("TRN2", target_bir_lowering=False, debug=False,
                   num_devices=NCORES)

    xT = nc.dram_tensor("xT", [E, T], bf16, kind="ExternalInput").ap()
    wq = nc.dram_tensor("wq", [E, ESL], bf16, kind="ExternalInput").ap()
    wk = nc.dram_tensor("wk", [E, ESL], bf16, kind="ExternalInput").ap()
    wv = nc.dram_tensor("wv", [E, ESL], bf16, kind="ExternalInput").ap()
    wp = nc.dram_tensor("wp", [ESL, E], bf16, kind="ExternalInput").ap()
    bias = nc.dram_tensor("bias", [P, 2 * NPAIR], f32,
                          kind="ExternalInput").ap()
    binmask = nc.dram_tensor("binmask", [P, 1280], bf16,
                             kind="ExternalInput").ap()
    out = nc.dram_tensor("out", [T, E], f32, kind="ExternalOutput").ap()

    with tile.TileContext(nc) as tc:
        _body(nc, tc, tile, mybir, bass,
              xT, wq, wk, wv, wp, bias, binmask, out)

    nc.compile()
    return nc


def _body(nc, tc, tile, mybir, bass,
          xT, wq, wk, wv, wp, bias, binmask, out):
    f32 = mybir.dt.float32
    bf16 = mybir.dt.bfloat16
    Exp = mybir.ActivationFunctionType.Exp

    cms = {}

    def open_pool(name, bufs, space=None, side=None):
        kw = {}
        if space:
            kw["space"] = space
        if side:
            kw["side"] = side
        cm = tc.tile_pool(name=name, bufs=bufs, **kw)
        pool = cm.__enter__()
        cms[id(pool)] = cm
        return pool

    def close_pool(pool):
        cms.pop(id(pool)).__exit__(None, None, None)

    # ---- pools ----------------------------------------------------------
    singles = open_pool("singles", 1)
    ps = open_pool("ps", 2, space="PSUM")
    pT_pool = open_pool("pT", 5)
    ob_pool = open_pool("ob", 2)
    bc_pool = open_pool("bc", 2)
    on_pool = open_pool("on", 2)
    ot_pool = open_pool("ot", 4)
    dr_pool = open_pool("dr", 2, space="DRAM")
    # right-stack: big resident tensors
    res_pool = open_pool("res", 1, side="right")

    # ---- resident loads --------------------------------------------------
    mask_t = singles.tile([P, 1280], bf16, tag="mask", name="mask")
    nc.sync.dma_start(out=mask_t[:], in_=binmask)
    bias_t = singles.tile([P, 2 * NPAIR], f32, tag="bias", name="bias")
    nc.sync.dma_start(out=bias_t[:], in_=bias)

    wq_c, wk_c, wv_c, wp_c = [], [], [], []
    for ec in range(NEC):
        t = res_pool.tile([P, ESL], bf16, tag="wv", name="wvc", bufs=NEC)
        nc.sync.dma_start(out=t[:], in_=wv[ec * P:(ec + 1) * P, :])
        wv_c.append(t)
    for ec in range(NEC):
        t = res_pool.tile([P, ESL], bf16, tag="wq", name="wqc", bufs=NEC)
        nc.sync.dma_start(out=t[:], in_=wq[ec * P:(ec + 1) * P, :])
        wq_c.append(t)
        t = res_pool.tile([P, ESL], bf16, tag="wk", name="wkc", bufs=NEC)
        nc.sync.dma_start(out=t[:], in_=wk[ec * P:(ec + 1) * P, :])
        wk_c.append(t)
    for c in range(NPAIR):
        t = res_pool.tile([P, E], bf16, tag="wp", name="wpc", bufs=NPAIR)
        nc.sync.dma_start(out=t[:], in_=wp[c * P:(c + 1) * P, :])
        wp_c.append(t)

    # x^T loaded in column chunks so compute starts after ~1/4 of the fill
    xr = [res_pool.tile([P, T], bf16, tag=f"xr{ec}", name=f"xr{ec}")
          for ec in range(NEC)]
    for ch in range(4):
        c0 = ch * (T // 4)
        for ec in range(NEC):
            nc.sync.dma_start(out=xr[ec][:, c0:c0 + T // 4],
                              in_=xT[ec * P:(ec + 1) * P, c0:c0 + T // 4])

    qT_t = [res_pool.tile([P, T], bf16, tag=f"qT{i}", name=f"qT{i}")
            for i in range(NPAIR)]
    kT_t = [res_pool.tile([P, T], bf16, tag=f"kT{i}", name=f"kT{i}")
            for i in range(NPAIR)]
    yT_t = [res_pool.tile([P, T], bf16, tag=f"yT{i}", name=f"yT{i}")
            for i in range(NPAIR)]
    # V per T-tile: per head [V(64) | ones] = 65-col stationary.
    v_t = [res_pool.tile([P, HPC, D + 1], bf16, tag=f"v{i}", name=f"v{i}")
           for i in range(NTT)]

    # ---- filler units (PE work interleaved between attention bins) ------
    def emit_v(tt):
        ts_ = slice(tt * P, (tt + 1) * P)
        psv = ps.tile([P, ESL], f32, tag="mm", name="psv", bufs=2)
        for ec in range(NEC):
            nc.tensor.matmul(psv[:], lhsT=xr[ec][:, ts_], rhs=wv_c[ec][:],
                             start=(ec == 0), stop=(ec == NEC - 1))
        nc.vector.memset(v_t[tt][:, :, D:D + 1], 1.0)
        nc.vector.tensor_copy(v_t[tt][:, :, 0:D], psv[:])

    def emit_qk(pt, tg):
        cs = slice(tg * TG, (tg + 1) * TG)
        pcol = slice(pt * P, (pt + 1) * P)
        psq = ps.tile([P, TG], f32, tag="mm", name="psq", bufs=2)
        for ec in range(NEC):
            nc.tensor.matmul(psq[:], lhsT=wq_c[ec][:, pcol],
                             rhs=xr[ec][:, cs],
                             start=(ec == 0), stop=(ec == NEC - 1))
        nc.vector.tensor_scalar_add(qT_t[pt][:, cs], psq[:],
                                    bias_t[:, pt:pt + 1])
        psk = ps.tile([P, TG], f32, tag="mm", name="psk", bufs=2)
        for ec in range(NEC):
            nc.tensor.matmul(psk[:], lhsT=wk_c[ec][:, pcol],
                             rhs=xr[ec][:, cs],
                             start=(ec == 0), stop=(ec == NEC - 1))
        nc.vector.tensor_scalar_add(kT_t[pt][:, cs], psk[:],
                                    bias_t[:, NPAIR + pt:NPAIR + pt + 1])

    def emit_proj(tt):
        ts_ = slice(tt * P, (tt + 1) * P)
        for ng in range(E // TG):
            pp = ps.tile([P, TG], f32, tag="mm", name="pp", bufs=2)
            for c in range(NPAIR):
                nc.tensor.matmul(pp[:], lhsT=yT_t[c][:, ts_],
                                 rhs=wp_c[c][:, ng * TG:(ng + 1) * TG],
                                 start=(c == 0), stop=(c == NPAIR - 1))
            ot = ot_pool.tile([P, TG], f32, tag="ot", name="ot")
            nc.vector.tensor_copy(ot[:], pp[:])
            nc.sync.dma_start(out=out[ts_, ng * TG:(ng + 1) * TG], in_=ot[:])

    # ---- attention for one (pair, q-group): both heads row-tiled ---------
    def emit_att(pt, qg, fillers):
        qb = qg * TG
        bins = _att_bins(qg)
        nbin = len(bins)
        pend = []  # (bin index, pT tiles per head, bin entries)
        blocks_total = sum(len(b) for b in bins)
        blk_idx = [0, 0]   # per-head running PV block index
        po = [ps.tile([P, TG], f32, tag="o", name=f"po{j}", bufs=2)
              for j in range(2)]

        def emit_pv(ent, pTs):
            for j in range(2):
                for (kt, c0, s0, n) in ent:
                    i = blk_idx[j]
                    nc.tensor.matmul(po[j][0:D + 1, s0:TG],
                                     lhsT=v_t[kt][:, 2 * pt + j, :],
                                     rhs=pTs[j][:, c0:c0 + n],
                                     start=(i == 0),
                                     stop=(i == blocks_total - 1))
                    blk_idx[j] += 1

        fill_iter = iter(fillers)
        for bi, ent in enumerate(bins):
            width = sum(n for (_, _, _, n) in ent)
            diag = qg > 0 and bi >= nbin - 2
            pss = [ps.tile([P, 2 * TG], f32, tag="s", name=f"pss{j}", bufs=2)
                   for j in range(2)]
            pTs = [pT_pool.tile([P, 2 * TG], bf16, tag="pT", name=f"pT{j}")
                   for j in range(2)]
            # S^T row-tiled: head j on array rows 64j..64j+63.
            for (kt, c0, s0, n) in ent:
                ks = slice(kt * P, (kt + 1) * P)
                qs = slice(qb + s0, qb + TG)
                for j in range(2):
                    rw = slice(64 * j, 64 * (j + 1))
                    nc.tensor.matmul(pss[j][:, c0:c0 + n],
                                     lhsT=kT_t[pt][rw, ks],
                                     rhs=qT_t[pt][rw, qs],
                                     start=True, stop=True)
            for j in range(2):
                nc.scalar.activation(pTs[j][:, 0:width], pss[j][:, 0:width],
                                     Exp, scale=0.125)
                if diag:
                    moff = 0 if bi == nbin - 2 else 896
                    nc.vector.tensor_mul(pTs[j][:, 0:width],
                                         pTs[j][:, 0:width],
                                         mask_t[:, moff:moff + width])
            pend.append((pTs, ent))
            # PV lags S by one bin so the PE isn't waiting on exp.
            if len(pend) > 1:
                ppTs, pent = pend.pop(0)
                emit_pv(pent, ppTs)
            for f in fill_iter:
                f()
                break
        ppTs, pent = pend.pop(0)
        emit_pv(pent, ppTs)
        for f in fill_iter:
            f()

        # ---- normalization (off the PE path) ----
        ob = ob_pool.tile([D + 1, 2, TG], f32, tag="ob", name="ob")
        for j in range(2):
            nc.vector.tensor_copy(ob[:, j, :], po[j][0:D + 1, :])
        den_d = dr_pool.tile([1, 2, TG], f32, tag="den", name="den")
        nc.gpsimd.dma_start(out=den_d[:], in_=ob[D:D + 1, :, :])
        bcast_in = bass.AP(
            tensor=den_d.tensor, offset=den_d.offset,
            ap=[[0, D]] + [list(a) for a in den_d.ap[1:]])
        bc = bc_pool.tile([D, 2, TG], f32, tag="bc", name="bc")
        nc.gpsimd.dma_start(out=bc[:], in_=bcast_in)
        rc = bc_pool.tile([D, 2, TG], f32, tag="rc", name="rc")
        nc.vector.reciprocal_approx_fast(out=rc[:], in_=bc[:])
        on = on_pool.tile([D, 2, TG], bf16, tag="on", name="on")
        nc.vector.tensor_mul(on[:], ob[0:D, :, :], rc[:])
        for j in range(2):
            nc.gpsimd.dma_start(
                out=yT_t[pt][64 * j:64 * (j + 1), qb:qb + TG],
                in_=on[:, j, :])

    # ---- schedule --------------------------------------------------------
    # Warm-up: V tiles 0-3 and pair-0 tg-0 QK, then attention with
    # remaining projection work interleaved as PE fillers.
    for tt in range(4):
        emit_v(tt)
    emit_qk(0, 0)

    fillers = {
        (0, 0): [lambda t=t: emit_v(t) for t in range(4, 8)] +
                [lambda: emit_qk(0, 1)],
        (0, 1): [lambda t=t: emit_v(t) for t in range(8, 12)] +
                [lambda: emit_qk(0, 2)],
        (0, 2): [lambda t=t: emit_v(t) for t in range(12, 16)] +
                [lambda: emit_qk(0, 3)],
        (0, 3): [lambda: emit_qk(1, 0), lambda: emit_qk(1, 1)],
        (1, 0): [lambda: emit_qk(1, 2), lambda: emit_qk(1, 3)],
        (1, 1): [lambda: emit_qk(2, 0), lambda: emit_qk(2, 1)],
        (1, 2): [lambda: emit_qk(2, 2), lambda: emit_qk(2, 3)],
        (1, 3): [lambda: emit_qk(3, 0), lambda: emit_qk(3, 1)],
        (2, 0): [lambda: emit_qk(3, 2), lambda: emit_qk(3, 3)],
        (3, 1): [lambda t=t: emit_proj(t) for t in range(0, 4)],
        (3, 2): [lambda t=t: emit_proj(t) for t in range(4, 8)],
        (3, 3): [lambda t=t: emit_proj(t) for t in range(8, 12)],
    }
    for pt in range(NPAIR):
        for qg in range(NTG):
            emit_att(pt, qg, fillers.get((pt, qg), []))
    for tt in range(12, 16):
        emit_proj(tt)

    close_pool(res_pool)
    close_pool(dr_pool)
    close_pool(ot_pool)
    close_pool(on_pool)
    close_pool(bc_pool)
    close_pool(ob_pool)
    close_pool(pT_pool)
    close_pool(ps)
    close_pool(singles)


def _get_program():
    if "nc" not in _CACHE:
        _CACHE["nc"] = _build_program()
    return _CACHE["nc"]


def make_in_maps(x, W_qkv, b_qkv, W_proj):
    """Per-core input dicts: core c -> (batch c%4, head-group c//4)."""
    import ml_dtypes
    x = np.asarray(x, np.float32)
    W_qkv = np.asarray(W_qkv, np.float32)
    b_qkv = np.asarray(b_qkv, np.float32)
    # Packed diagonal-bin mask: segments tri(512)|tri(384)|tri(256)|tri(128);
    # tri(n)[p, j] = (j >= p) for j in [0, n).
    segs = [512, 384, 256, 128]
    binmask = np.zeros((P, sum(segs)), np.float32)
    off = 0
    for n in segs:
        binmask[:, off:off + n] = (np.arange(n)[None, :] >=
                                   np.arange(P)[:, None])
        off += n
    cvt = lambda a: np.ascontiguousarray(a).astype(ml_dtypes.bfloat16)
    in_maps = []
    for c in range(NCORES):
        b, g = c % B, c // B
        gs = slice(g * ESL, (g + 1) * ESL)
        bqs = b_qkv[0 * E:1 * E][gs]
        bks = b_qkv[1 * E:2 * E][gs]
        bias = np.zeros((P, 2 * NPAIR), np.float32)
        for pt in range(NPAIR):
            bias[:, pt] = bqs[pt * P:(pt + 1) * P]
            bias[:, NPAIR + pt] = bks[pt * P:(pt + 1) * P]
        in_maps.append({
            "xT": cvt(x[b].T),
            "wq": cvt(W_qkv[:, 0 * E:1 * E][:, gs]),
            "wk": cvt(W_qkv[:, 1 * E:2 * E][:, gs]),
            "wv": cvt(W_qkv[:, 2 * E:3 * E][:, gs]),
            "wp": cvt(np.asarray(W_proj, np.float32)[gs, :]),
            "bias": np.ascontiguousarray(bias),
            "binmask": cvt(binmask),
        })
    return in_maps


def gather_output(results, b_qkv, b_proj, W_proj):
    """Sum the two row-parallel partials per batch; fold v/proj biases."""
    b_qkv = np.asarray(b_qkv, np.float64)
    W_proj = np.asarray(W_proj, np.float64)
    b_v = b_qkv[2 * E:3 * E]
    const = b_v @ W_proj + np.asarray(b_proj, np.float64)
    out = np.empty((B, T, E), np.float32)
    for b in range(B):
        out[b] = (results[b]["out"].astype(np.float64) +
                  results[b + B]["out"].astype(np.float64) +
                  const).astype(np.float32)
    return out


def run_on_hw(inputs, trace=False, **kwargs):
    from concourse.bass_utils import run_bass_kernel_spmd
    nc = _get_program()
    in_maps = make_in_maps(inputs["x"], inputs["W_qkv"], inputs["b_qkv"],
                           inputs["W_proj"])
    res = run_bass_kernel_spmd(nc, in_maps, list(range(NCORES)), trace=trace,
                               **kwargs)
    out = gather_output(res.results, inputs["b_qkv"], inputs["b_proj"],
                        inputs["W_proj"])
    return out, res


def kernel(x, W_qkv, b_qkv, W_proj, b_proj):
    out, _ = run_on_hw({"x": x, "W_qkv": W_qkv, "b_qkv": b_qkv,
                        "W_proj": W_proj, "b_proj": b_proj})
    return out


# revision 7
# speedup vs baseline: 1.1239x; 1.0467x over previous
"""Trainium2 Bass kernel for nn_CausalSelfAttention_6442450944521.

Sparse-attention causal self-attention block:
  B=4, T=2048 (rows<512: full attention over cols<512; rows>=512: causal),
  E=1024, H=16, D=64.

Sharding: batch (4) x head-group (2 groups of 8 heads) across 8 cores.
Each core computes qkv^T projections, block-sparse attention via S^T = K Q^T
tiles, and its row-slice of the output projection; the two head-group
partials per batch are summed on the host (row-parallel tensor parallelism).

v2 structure:
  - S^T matmuls are ROW-TILED: the two heads of a pair run concurrently on
    the 128x128 PE array (head A rows 0-63 via tile_position (0,0), head B
    rows 64-127 via (64,0)) since the contraction dim is only D=64. kT/qT
    are stored pair-stacked [128, T] so base-partition slicing infers the
    tile positions; no zero padding or memsets needed.
  - PV keeps the k-contraction layout with a 65-col stationary [V|ones]
    (ones column yields the softmax denominator for free); no 128-padding.
  - S blocks land in [128, 1024] two-bank PSUM bins; ONE exp per bin on the
    scalar engine (per-op ACT overhead was pacing the attention phase).
  - Diagonal-block masks are applied with one host-packed mask tile.
  - Q/K bias adds + PSUM evacuations run on the vector engine.
  - Softmax normalization: denominator row -> DRAM -> partition-broadcast
    back (gpsimd DMA queue), reciprocal+multiply on DVE, off the PE path.
  - Projection (V/QK/out-proj) matmul units are interleaved between
    attention bins in PE program order so the PE fills scalar-exp waits.
"""

import os
import sys

if "/opt/trn_rl_repo" not in sys.path:
    sys.path.insert(0, "/opt/trn_rl_repo")

import numpy as np

# Problem constants (hardcoded per harness contract).
B = 4
T = 2048
E = 1024
H = 16
D = 64
NCORES = 8
HPC = H // 2          # heads per core = 8
ESL = HPC * D         # per-core E-slice = 512
P = 128               # SBUF/PSUM partitions
TG = 512              # q-group width
NTG = T // TG         # 4
NTT = T // P          # 16
NEC = E // P          # 8 contraction chunks over E
NPAIR = HPC // 2      # 4 head-pair tiles

_CACHE = {}


def _att_bins(qg):
    """Bins of S^T blocks for q-group qg. Each bin is a list of
    (kt, c0, s0, n): k-tile index, column offset in the [128,1024] bin,
    q-offset within the group, and width. Total bin width <= 1024."""
    bins = []
    nf = 4 if qg == 0 else 4 * qg
    for k0 in range(0, nf, 2):
        bins.append([(k0, 0, 0, TG), (k0 + 1, TG, 0, TG)])
    if qg > 0:
        m0 = 4 * qg
        bins.append([(m0, 0, 0, 512), (m0 + 1, 512, 128, 384)])
        bins.append([(m0 + 2, 0, 256, 256), (m0 + 3, 256, 384, 128)])
    return bins


def _build_program():
    import concourse.bass as bass
    import concourse.tile as tile
    from concourse import bacc, mybir

    f32 = mybir.dt.float32
    bf16 = mybir.dt.bfloat16

    nc = bacc.Bacc("TRN2", target_bir_lowering=False, debug=False,
                   num_devices=NCORES)

    xT = nc.dram_tensor("xT", [E, T], bf16, kind="ExternalInput").ap()
    wq = nc.dram_tensor("wq", [E, ESL], bf16, kind="ExternalInput").ap()
    wk = nc.dram_tensor("wk", [E, ESL], bf16, kind="ExternalInput").ap()
    wv = nc.dram_tensor("wv", [E, ESL], bf16, kind="ExternalInput").ap()
    wp = nc.dram_tensor("wp", [ESL, E], bf16, kind="ExternalInput").ap()
    bias = nc.dram_tensor("bias", [P, 2 * NPAIR], f32,
                          kind="ExternalInput").ap()
    binmask = nc.dram_tensor("binmask", [P, 1280], bf16,
                             kind="ExternalInput").ap()
    out = nc.dram_tensor("out", [T, E], f32, kind="ExternalOutput").ap()

    with tile.TileContext(nc) as tc:
        _body(nc, tc, tile, mybir, bass,
              xT, wq, wk, wv, wp, bias, binmask, out)

    nc.compile()
    return nc


def _body(nc, tc, tile, mybir, bass,
          xT, wq, wk, wv, wp, bias, binmask, out):
    f32 = mybir.dt.float32
    bf16 = mybir.dt.bfloat16
    Exp = mybir.ActivationFunctionType.Exp

    cms = {}

    def open_pool(name, bufs, space=None, side=None):
        kw = {}
        if space:
            kw["space"] = space
        if side:
            kw["side"] = side
        cm = tc.tile_pool(name=name, bufs=bufs, **kw)
        pool = cm.__enter__()
        cms[id(pool)] = cm
        return pool

    def close_pool(pool):
        cms.pop(id(pool)).__exit__(None, None, None)

    # ---- pools ----------------------------------------------------------
    singles = open_pool("singles", 1)
    ps = open_pool("ps", 2, space="PSUM")
    pT_pool = open_pool("pT", 5)
    ob_pool = open_pool("ob", 2)
    bc_pool = open_pool("bc", 2)
    on_pool = open_pool("on", 2)
    ot_pool = open_pool("ot", 4)
    dr_pool = open_pool("dr", 2, space="DRAM")
    # right-stack: big resident tensors
    res_pool = open_pool("res", 1, side="right")

    # ---- resident loads --------------------------------------------------
    mask_t = singles.tile([P, 1280], bf16, tag="mask", name="mask")
    nc.sync.dma_start(out=mask_t[:], in_=binmask)
    bias_t = singles.tile([P, 2 * NPAIR], f32, tag="bias", name="bias")
    nc.sync.dma_start(out=bias_t[:], in_=bias)

    # Load order: x chunk 0 + wv first (V compute starts ~8us in), then
    # wq/wk (pair-0 projections), then the rest of x, then wp.
    xr = [res_pool.tile([P, T], bf16, tag=f"xr{ec}", name=f"xr{ec}")
          for ec in range(NEC)]
    for ec in range(NEC):
        nc.sync.dma_start(out=xr[ec][:, 0:T // 4],
                          in_=xT[ec * P:(ec + 1) * P, 0:T // 4])
    wq_c, wk_c, wv_c, wp_c = [], [], [], []
    for ec in range(NEC):
        t = res_pool.tile([P, ESL], bf16, tag="wv", name="wvc", bufs=NEC)
        nc.sync.dma_start(out=t[:], in_=wv[ec * P:(ec + 1) * P, :])
        wv_c.append(t)
    for ec in range(NEC):
        t = res_pool.tile([P, ESL], bf16, tag="wq", name="wqc", bufs=NEC)
        nc.sync.dma_start(out=t[:], in_=wq[ec * P:(ec + 1) * P, :])
        wq_c.append(t)
        t = res_pool.tile([P, ESL], bf16, tag="wk", name="wkc", bufs=NEC)
        nc.sync.dma_start(out=t[:], in_=wk[ec * P:(ec + 1) * P, :])
        wk_c.append(t)
    for ch in range(1, 4):
        c0 = ch * (T // 4)
        for ec in range(NEC):
            nc.sync.dma_start(out=xr[ec][:, c0:c0 + T // 4],
                              in_=xT[ec * P:(ec + 1) * P, c0:c0 + T // 4])
    for c in range(NPAIR):
        t = res_pool.tile([P, E], bf16, tag="wp", name="wpc", bufs=NPAIR)
        nc.sync.dma_start(out=t[:], in_=wp[c * P:(c + 1) * P, :])
        wp_c.append(t)

    qT_t = [res_pool.tile([P, T], bf16, tag=f"qT{i}", name=f"qT{i}")
            for i in range(NPAIR)]
    kT_t = [res_pool.tile([P, T], bf16, tag=f"kT{i}", name=f"kT{i}")
            for i in range(NPAIR)]
    yT_t = [res_pool.tile([P, T], bf16, tag=f"yT{i}", name=f"yT{i}")
            for i in range(NPAIR)]
    # V per T-tile: per head [V(64) | ones | zeros(63)] = full 128-col
    # stationary (FWL-eligible). Zero/ones fills run during the input-DMA
    # dead time at program start.
    v_t = [res_pool.tile([P, HPC, P], bf16, tag=f"v{i}", name=f"v{i}")
           for i in range(NTT)]
    for tt in range(NTT):
        nc.vector.memset(v_t[tt][:, :, D + 1:], 0.0)
        nc.vector.memset(v_t[tt][:, :, D:D + 1], 1.0)

    # ---- filler units (PE work interleaved between attention bins) ------
    def emit_v(tt):
        ts_ = slice(tt * P, (tt + 1) * P)
        psv = ps.tile([P, ESL], f32, tag="mm", name="psv", bufs=2)
        for ec in range(NEC):
            nc.tensor.matmul(psv[:], lhsT=xr[ec][:, ts_], rhs=wv_c[ec][:],
                             start=(ec == 0), stop=(ec == NEC - 1))
        nc.vector.tensor_copy(v_t[tt][:, :, 0:D], psv[:])

    def emit_qk(pt, tg):
        cs = slice(tg * TG, (tg + 1) * TG)
        pcol = slice(pt * P, (pt + 1) * P)
        psq = ps.tile([P, TG], f32, tag="mm", name="psq", bufs=2)
        for ec in range(NEC):
            nc.tensor.matmul(psq[:], lhsT=wq_c[ec][:, pcol],
                             rhs=xr[ec][:, cs],
                             start=(ec == 0), stop=(ec == NEC - 1))
        nc.vector.tensor_scalar_add(qT_t[pt][:, cs], psq[:],
                                    bias_t[:, pt:pt + 1])
        psk = ps.tile([P, TG], f32, tag="mm", name="psk", bufs=2)
        for ec in range(NEC):
            nc.tensor.matmul(psk[:], lhsT=wk_c[ec][:, pcol],
                             rhs=xr[ec][:, cs],
                             start=(ec == 0), stop=(ec == NEC - 1))
        nc.vector.tensor_scalar_add(kT_t[pt][:, cs], psk[:],
                                    bias_t[:, NPAIR + pt:NPAIR + pt + 1])

    def emit_proj(tt):
        ts_ = slice(tt * P, (tt + 1) * P)
        for ng in range(E // TG):
            pp = ps.tile([P, TG], f32, tag="mm", name="pp", bufs=2)
            for c in range(NPAIR):
                nc.tensor.matmul(pp[:], lhsT=yT_t[c][:, ts_],
                                 rhs=wp_c[c][:, ng * TG:(ng + 1) * TG],
                                 start=(c == 0), stop=(c == NPAIR - 1))
            ot = ot_pool.tile([P, TG], f32, tag="ot", name="ot")
            nc.vector.tensor_copy(ot[:], pp[:])
            nc.sync.dma_start(out=out[ts_, ng * TG:(ng + 1) * TG], in_=ot[:])

    # ---- attention for one (pair, q-group): both heads row-tiled ---------
    def emit_att(pt, qg, fillers):
        qb = qg * TG
        bins = _att_bins(qg)
        nbin = len(bins)
        pend = []  # (bin index, pT tiles per head, bin entries)
        blocks_total = sum(len(b) for b in bins)
        blk_idx = [0, 0]   # per-head running PV block index
        po = [ps.tile([P, TG], f32, tag="o", name=f"po{j}", bufs=2)
              for j in range(2)]

        def emit_pv(ent, pTs):
            for j in range(2):
                for (kt, c0, s0, n) in ent:
                    i = blk_idx[j]
                    nc.tensor.matmul(po[j][:, s0:TG],
                                     lhsT=v_t[kt][:, 2 * pt + j, :],
                                     rhs=pTs[j][:, c0:c0 + n],
                                     start=(i == 0),
                                     stop=(i == blocks_total - 1))
                    blk_idx[j] += 1

        fill_iter = iter(fillers)
        for bi, ent in enumerate(bins):
            width = sum(n for (_, _, _, n) in ent)
            diag = qg > 0 and bi >= nbin - 2
            pss = [ps.tile([P, 2 * TG], f32, tag="s", name=f"pss{j}", bufs=2)
                   for j in range(2)]
            pTs = [pT_pool.tile([P, 2 * TG], bf16, tag="pT", name=f"pT{j}")
                   for j in range(2)]
            # S^T row-tiled: head j on array rows 64j..64j+63.
            for (kt, c0, s0, n) in ent:
                ks = slice(kt * P, (kt + 1) * P)
                qs = slice(qb + s0, qb + TG)
                for j in range(2):
                    rw = slice(64 * j, 64 * (j + 1))
                    nc.tensor.matmul(pss[j][:, c0:c0 + n],
                                     lhsT=kT_t[pt][rw, ks],
                                     rhs=qT_t[pt][rw, qs],
                                     start=True, stop=True)
            for j in range(2):
                nc.scalar.activation(pTs[j][:, 0:width], pss[j][:, 0:width],
                                     Exp, scale=0.125)
                if diag:
                    moff = 0 if bi == nbin - 2 else 896
                    nc.vector.tensor_mul(pTs[j][:, 0:width],
                                         pTs[j][:, 0:width],
                                         mask_t[:, moff:moff + width])
            pend.append((pTs, ent))
            # PV lags S by one bin so the PE isn't waiting on exp.
            if len(pend) > 1:
                ppTs, pent = pend.pop(0)
                emit_pv(pent, ppTs)
            for f in fill_iter:
                f()
                break
        ppTs, pent = pend.pop(0)
        emit_pv(pent, ppTs)
        for f in fill_iter:
            f()

        # ---- normalization (off the PE path) ----
        ob = ob_pool.tile([D + 1, 2, TG], f32, tag="ob", name="ob")
        for j in range(2):
            nc.vector.tensor_copy(ob[:, j, :], po[j][0:D + 1, :])
        den_d = dr_pool.tile([1, 2, TG], f32, tag="den", name="den")
        nc.gpsimd.dma_start(out=den_d[:], in_=ob[D:D + 1, :, :])
        bcast_in = bass.AP(
            tensor=den_d.tensor, offset=den_d.offset,
            ap=[[0, D]] + [list(a) for a in den_d.ap[1:]])
        bc = bc_pool.tile([D, 2, TG], f32, tag="bc", name="bc")
        nc.gpsimd.dma_start(out=bc[:], in_=bcast_in)
        rc = bc_pool.tile([D, 2, TG], f32, tag="rc", name="rc")
        nc.vector.reciprocal_approx_fast(out=rc[:], in_=bc[:])
        on = on_pool.tile([D, 2, TG], bf16, tag="on", name="on")
        nc.vector.tensor_mul(on[:], ob[0:D, :, :], rc[:])
        for j in range(2):
            nc.gpsimd.dma_start(
                out=yT_t[pt][64 * j:64 * (j + 1), qb:qb + TG],
                in_=on[:, j, :])

    # ---- schedule --------------------------------------------------------
    # Warm-up: V tiles 0-3 and pair-0 tg-0 QK, then attention with
    # remaining projection work interleaved as PE fillers.
    for tt in range(4):
        emit_v(tt)
    emit_qk(0, 0)

    fillers = {
        (0, 0): [lambda: emit_qk(0, 1)] +
                [lambda t=t: emit_v(t) for t in range(4, 8)],
        (0, 1): [lambda: emit_qk(0, 2)] +
                [lambda t=t: emit_v(t) for t in range(8, 12)],
        (0, 2): [lambda: emit_qk(0, 3)] +
                [lambda t=t: emit_v(t) for t in range(12, 16)],
        (0, 3): [lambda: emit_qk(1, 0), lambda: emit_qk(1, 1)],
        (1, 0): [lambda: emit_qk(1, 2), lambda: emit_qk(1, 3)],
        (1, 1): [lambda: emit_qk(2, 0), lambda: emit_qk(2, 1)],
        (1, 2): [lambda: emit_qk(2, 2), lambda: emit_qk(2, 3)],
        (1, 3): [lambda: emit_qk(3, 0), lambda: emit_qk(3, 1)],
        (2, 0): [lambda: emit_qk(3, 2), lambda: emit_qk(3, 3)],
        (3, 2): [lambda t=t: emit_proj(t) for t in range(4, 8)],
        (3, 3): [lambda t=t: emit_proj(t) for t in range(8, 12)],
        (3, 0): [lambda t=t: emit_proj(t) for t in range(12, 16)],
    }
    # Pair 3 runs q-groups 1,2,3 then 0 so each completed q-group's output
    # projection fills the next attention unit; the tail is only qg 0's
    # projections.
    qg_order = {3: [1, 2, 3, 0]}
    for pt in range(NPAIR):
        for qg in qg_order.get(pt, range(NTG)):
            emit_att(pt, qg, fillers.get((pt, qg), []))
    for tt in range(0, 4):
        emit_proj(tt)

    close_pool(res_pool)
    close_pool(dr_pool)
    close_pool(ot_pool)
    close_pool(on_pool)
    close_pool(bc_pool)
    close_pool(ob_pool)
    close_pool(pT_pool)
    close_pool(ps)
    close_pool(singles)


def _get_program():
    if "nc" not in _CACHE:
        _CACHE["nc"] = _build_program()
    return _CACHE["nc"]


def make_in_maps(x, W_qkv, b_qkv, W_proj):
    """Per-core input dicts: core c -> (batch c%4, head-group c//4)."""
    import ml_dtypes
    x = np.asarray(x, np.float32)
    W_qkv = np.asarray(W_qkv, np.float32)
    b_qkv = np.asarray(b_qkv, np.float32)
    # Packed diagonal-bin mask: segments tri(512)|tri(384)|tri(256)|tri(128);
    # tri(n)[p, j] = (j >= p) for j in [0, n).
    segs = [512, 384, 256, 128]
    binmask = np.zeros((P, sum(segs)), np.float32)
    off = 0
    for n in segs:
        binmask[:, off:off + n] = (np.arange(n)[None, :] >=
                                   np.arange(P)[:, None])
        off += n
    cvt = lambda a: np.ascontiguousarray(a).astype(ml_dtypes.bfloat16)
    in_maps = []
    for c in range(NCORES):
        b, g = c % B, c // B
        gs = slice(g * ESL, (g + 1) * ESL)
        bqs = b_qkv[0 * E:1 * E][gs]
        bks = b_qkv[1 * E:2 * E][gs]
        bias = np.zeros((P, 2 * NPAIR), np.float32)
        for pt in range(NPAIR):
            bias[:, pt] = bqs[pt * P:(pt + 1) * P]
            bias[:, NPAIR + pt] = bks[pt * P:(pt + 1) * P]
        in_maps.append({
            "xT": cvt(x[b].T),
            "wq": cvt(W_qkv[:, 0 * E:1 * E][:, gs]),
            "wk": cvt(W_qkv[:, 1 * E:2 * E][:, gs]),
            "wv": cvt(W_qkv[:, 2 * E:3 * E][:, gs]),
            "wp": cvt(np.asarray(W_proj, np.float32)[gs, :]),
            "bias": np.ascontiguousarray(bias),
            "binmask": cvt(binmask),
        })
    return in_maps


def gather_output(results, b_qkv, b_proj, W_proj):
    """Sum the two row-parallel partials per batch; fold v/proj biases."""
    b_qkv = np.asarray(b_qkv, np.float64)
    W_proj = np.asarray(W_proj, np.float64)
    b_v = b_qkv[2 * E:3 * E]
    const = b_v @ W_proj + np.asarray(b_proj, np.float64)
    out = np.empty((B, T, E), np.float32)
    for b in range(B):
        out[b] = (results[b]["out"].astype(np.float64) +
                  results[b + B]["out"].astype(np.float64) +
                  const).astype(np.float32)
    return out


def run_on_hw(inputs, trace=False, **kwargs):
    from concourse.bass_utils import run_bass_kernel_spmd
    nc = _get_program()
    in_maps = make_in_maps(inputs["x"], inputs["W_qkv"], inputs["b_qkv"],
                           inputs["W_proj"])
    res = run_bass_kernel_spmd(nc, in_maps, list(range(NCORES)), trace=trace,
                               **kwargs)
    out = gather_output(res.results, inputs["b_qkv"], inputs["b_proj"],
                        inputs["W_proj"])
    return out, res


def kernel(x, W_qkv, b_qkv, W_proj, b_proj):
    out, _ = run_on_hw({"x": x, "W_qkv": W_qkv, "b_qkv": b_qkv,
                        "W_proj": W_proj, "b_proj": b_proj})
    return out
